# revision 87
# baseline (speedup 1.0000x reference)
"""Trainium2 Bass kernel for nn_ChunkedMultiHeadCardPassingLayer.

Sharding: 8 cores = (batch b = core//2) x (T-half = core%2). Each core
processes 2048 contiguous tokens of one batch end-to-end; the only
cross-core dependency is the chunk-carry prefix, resolved with a 4KB
paired AllReduce.

Restructure vs the original baseline (596us -> ~495us):
- all matmul operands are 2-byte (bf16) -> cheap LDWEIGHTS, less DMA
- x kept transposed + resident in SBUF for phases A and D
- local_cum kept in SBUF as bf16 (no DRAM spill round-trip)
- chunk sums extracted from cumsum row 127 via tiny DMA (csel dropped)
- ncarry broadcast via selector-stationary matmul (no bounce DMAs)
- cards transposed via blocked XBAR DMA-transpose, 2 calls per chunk
  (no PE transposes, no PSUM pressure, no evac copies)
- MLP tail uses matmul linearity: o2 = W2.T@hb + (alpha*W2).T@(hb*e3),
  removing an elementwise pass; b1/b2 folded into downstream biases
- activation chain spread across scalar/vector/gpsimd; same-function
  scalar ops batched to limit ACT table reloads
- software pipeline at distance 2 (C(pg) | D(pg-1) | E(pg-2)) with a
  static unit interleave so E's dense matmul bursts keep the PE busy
  while D's activation chain and C's LN chain are in flight
"""
import os
os.environ.setdefault("JAX_PLATFORMS", "cpu")

import numpy as np
import ml_dtypes
from contextlib import ExitStack

import concourse.bacc as bacc
import concourse.mybir as mybir
import concourse.tile as tile
from concourse.bass_utils import run_bass_kernel_spmd

F32 = mybir.dt.float32
F32R = mybir.dt.float32r
BF16 = mybir.dt.bfloat16
AX = mybir.AxisListType
ALU = mybir.AluOpType
ACTF = mybir.ActivationFunctionType

# problem constants
B, T, C = 4, 4096, 1024
H, CS = 16, 128
D = C // H            # 64
NCORES = 8
R = T // 2            # 2048 rows per core
NCH = R // CS         # 16 chunks per core
NG = C // 128         # 8 groups of (2 heads x 64)
NPG = NCH // 4        # 4 position groups of 512
EPS = 1e-5
P = 128
HH = 8                # heads per 512 half


def _build(ncores, alpha, has_mkb, has_gtb, has_pjb,
           has_carry_gb, has_ln_g, has_ln_b):
    nc = bacc.Bacc("TRN2", target_bir_lowering=False, debug=False,
                   num_devices=ncores)

    # ---------------- DRAM I/O ----------------
    xt_d = nc.dram_tensor("xt", [C, R], BF16, kind="ExternalInput")
    xn_d = nc.dram_tensor("xn", [R, C], F32, kind="ExternalInput")
    mkw_d = nc.dram_tensor("mkw", [C, C], BF16, kind="ExternalInput")
    gtw_d = nc.dram_tensor("gtw", [C, C], BF16, kind="ExternalInput")
    pjw_d = nc.dram_tensor("pjw", [C, C], BF16, kind="ExternalInput")
    mkb_d = nc.dram_tensor("mkb", [1, C], BF16, kind="ExternalInput")
    gtb_d = nc.dram_tensor("gtb", [1, C], BF16, kind="ExternalInput")
    pjb_d = nc.dram_tensor("pjb", [1, C], BF16, kind="ExternalInput")
    w1x_d = nc.dram_tensor("w1x", [2 * D, 2 * D], BF16, kind="ExternalInput")
    w1c_d = nc.dram_tensor("w1c", [2 * D, 2 * D], BF16, kind="ExternalInput")
    b1_d = nc.dram_tensor("b1c", [2 * D, 1], F32, kind="ExternalInput")
    w2_d = nc.dram_tensor("w2", [2 * D, D], BF16, kind="ExternalInput")
    w2a_d = nc.dram_tensor("w2a", [2 * D, D], BF16, kind="ExternalInput")
    ut_d = nc.dram_tensor("ut", [P, P], BF16, kind="ExternalInput")
    st_d = nc.dram_tensor("st", [P, P], BF16, kind="ExternalInput")
    l0_d = nc.dram_tensor("l0", [NCH, NCH], BF16, kind="ExternalInput")
    onesr_d = nc.dram_tensor("onesr", [1, P], BF16, kind="ExternalInput")
    selb_d = nc.dram_tensor("selb", [NCH, NCH * P], BF16,
                            kind="ExternalInput")
    segm_d = nc.dram_tensor("segm", [1, 1], F32, kind="ExternalInput")
    usem_d = nc.dram_tensor("usem", [1, 1], F32, kind="ExternalInput")
    cgr_d = nc.dram_tensor("cgr", [NCH, D], F32, kind="ExternalInput")
    cbr_d = nc.dram_tensor("cbr", [NCH, D], F32, kind="ExternalInput")
    lgr_d = nc.dram_tensor("lgr", [P, C], F32, kind="ExternalInput")
    lbr_d = nc.dram_tensor("lbr", [P, C], F32, kind="ExternalInput")

    y_d = nc.dram_tensor("y", [R, C], F32, kind="ExternalOutput")

    cc_in = nc.dram_tensor("cc_in", [1, C], F32)
    cc_out = nc.dram_tensor("cc_out", [1, C], F32)

    groups = ([[i, i + 1] for i in range(0, ncores, 2)]
              if ncores > 1 else [[0]])

    with tile.TileContext(nc) as tc, ExitStack() as top:
        const_p = top.enter_context(tc.tile_pool(name="const", bufs=1))
        xt_p = top.enter_context(tc.tile_pool(name="xtp", bufs=1))
        lc_p = top.enter_context(tc.tile_pool(name="lcp", bufs=1))
        carr_p = top.enter_context(tc.tile_pool(name="carr", bufs=1))

        # ---------- constants ----------
        ut = const_p.tile([P, P], BF16)
        st = const_p.tile([P, P], BF16)
        l0 = const_p.tile([NCH, NCH], BF16)
        w1x = const_p.tile([2 * D, 2 * D], BF16)
        w1c = const_p.tile([2 * D, 2 * D], BF16)
        b1c = const_p.tile([2 * D, 1], F32)
        w2 = const_p.tile([2 * D, D], BF16)
        w2a = const_p.tile([2 * D, D], BF16)
        segm = const_p.tile([1, 1], F32)
        usem = const_p.tile([1, 1], F32)
        ones1r = const_p.tile([1, P], BF16)
        selb = const_p.tile([NCH, NCH * P], BF16)
        for t_, d_ in ((ut, ut_d), (st, st_d), (l0, l0_d),
                       (w1x, w1x_d), (w1c, w1c_d), (b1c, b1_d),
                       (w2, w2_d), (w2a, w2a_d), (segm, segm_d),
                       (usem, usem_d), (ones1r, onesr_d), (selb, selb_d)):
            nc.sync.dma_start(t_[:], d_.ap())
        ones16_1 = const_p.tile([NCH, 1], BF16)
        nc.vector.memset(ones16_1[:], 1.0)
        ones1_16 = const_p.tile([1, NCH], BF16)
        nc.vector.memset(ones1_16[:], 1.0)
        eps128 = const_p.tile([P, 1], F32)
        nc.vector.memset(eps128[:], EPS)
        eps16 = const_p.tile([NCH, 1], F32)
        nc.vector.memset(eps16[:], EPS)
        if has_mkb or has_gtb:
            mkb = const_p.tile([1, C], BF16)
            gtb = const_p.tile([1, C], BF16)
            nc.sync.dma_start(mkb[:], mkb_d.ap())
            nc.sync.dma_start(gtb[:], gtb_d.ap())
        if has_pjb:
            pjb = const_p.tile([1, C], BF16)
            nc.sync.dma_start(pjb[:], pjb_d.ap())
        if has_carry_gb:
            cgr = const_p.tile([NCH, D], F32)
            cbr = const_p.tile([NCH, D], F32)
            nc.sync.dma_start(cgr[:], cgr_d.ap())
            nc.sync.dma_start(cbr[:], cbr_d.ap())

        # resident x (transposed), two tiles per chan-group (half-R each)
        xth = [[xt_p.tile([P, R // 2], BF16, tag=f"xt{g}_{hf}",
                          name=f"xt{g}_{hf}") for hf in range(2)]
               for g in range(NG)]
        # resident pjw (loaded later; pool allocated at top level)
        pjw_p = top.enter_context(tc.tile_pool(name="pjp", bufs=1))
        pjw = [pjw_p.tile([P, C], BF16, tag=f"pj{k}", name=f"pj{k}")
               for k in range(NG)]
        lgr = pjw_p.tile([P, C], F32) if has_ln_g else None
        lbr = pjw_p.tile([P, C], F32) if has_ln_b else None

        # resident local_cum (bf16) + chunk sums + normalized carries
        lc_sb = []
        for j in range(NCH):
            t_ = lc_p.tile([P, C], BF16, tag=f"lc{j}", name=f"lc{j}")
            lc_sb.append(t_)
        cs_sb = carr_p.tile([NCH, C], BF16)
        ncarry = carr_p.tile([NCH, C], BF16)

        # ================ phase A: pm/gate/scan ================
        with tc.tile_pool(name="wgt", bufs=1) as wgt_p, \
             tc.tile_pool(name="ph1", bufs=2) as ph1_p, \
             tc.tile_pool(name="psA", bufs=1, space="PSUM") as psA_p, \
             tc.tile_pool(name="pslc", bufs=2, space="PSUM") as pslc_p:
            mkw, gtw = [], []
            for k in range(NG):
                mt = wgt_p.tile([P, C], BF16, tag=f"mk{k}", name=f"mk{k}")
                gt_ = wgt_p.tile([P, C], BF16, tag=f"gk{k}", name=f"gk{k}")
                nc.sync.dma_start(mt[:], mkw_d.ap()[k * P:(k + 1) * P, :])
                nc.sync.dma_start(gt_[:], gtw_d.ap()[k * P:(k + 1) * P, :])
                mkw.append(mt)
                gtw.append(gt_)
            for hf in range(2):
                for g in range(NG):
                    nc.sync.dma_start(
                        xth[g][hf][:],
                        xt_d.ap()[g * P:(g + 1) * P,
                                  hf * (R // 2):(hf + 1) * (R // 2)])
            for k in range(NG):
                nc.sync.dma_start(pjw[k][:], pjw_d.ap()[k * P:(k + 1) * P, :])
            if has_ln_g:
                nc.sync.dma_start(lgr[:], lgr_d.ap())
            if has_ln_b:
                nc.sync.dma_start(lbr[:], lbr_d.ap())
            for j in range(NCH):
                pm0 = psA_p.tile([P, 512], F32, tag="pm0", name="pm0")
                gt0 = psA_p.tile([P, 512], F32, tag="gt0", name="gt0")
                pm1 = psA_p.tile([P, 512], F32, tag="pm1", name="pm1")
                gt1 = psA_p.tile([P, 512], F32, tag="gt1", name="gt1")
                s0, s1_ = slice(0, 512), slice(512, 1024)
                jh, jc = j // 8, (j % 8) * P
                for k in range(NG):
                    lhs = xth[k][jh][:, jc:jc + P]
                    st_ = (k == 0)
                    spm = (k == NG - 1) and not has_mkb
                    spg = (k == NG - 1) and not has_gtb
                    nc.tensor.matmul(pm0[:], lhs, mkw[k][:, s0],
                                     start=st_, stop=spm)
                    nc.tensor.matmul(gt0[:], lhs, gtw[k][:, s0],
                                     start=st_, stop=spg)
                    nc.tensor.matmul(pm1[:], lhs, mkw[k][:, s1_],
                                     start=st_, stop=spm)
                    nc.tensor.matmul(gt1[:], lhs, gtw[k][:, s1_],
                                     start=st_, stop=spg)
                if has_mkb:
                    nc.tensor.matmul(pm0[:], ones1r[:], mkb[:, s0],
                                     start=False, stop=True)
                    nc.tensor.matmul(pm1[:], ones1r[:], mkb[:, s1_],
                                     start=False, stop=True)
                if has_gtb:
                    nc.tensor.matmul(gt0[:], ones1r[:], gtb[:, s0],
                                     start=False, stop=True)
                    nc.tensor.matmul(gt1[:], ones1r[:], gtb[:, s1_],
                                     start=False, stop=True)
                gated = []
                for n, (pm_ps, gt_ps) in enumerate(((pm0, gt0), (pm1, gt1))):
                    gates = ph1_p.tile([P, 512], F32, tag=f"gates{n}",
                                       name=f"gates{n}")
                    nc.scalar.activation(gates[:], gt_ps[:], ACTF.Sigmoid)
                    gd = ph1_p.tile([P, 512], BF16, tag=f"gated{n}",
                                    name=f"gated{n}")
                    nc.vector.tensor_tensor(gd[:], gates[:], pm_ps[:],
                                            op=ALU.mult)
                    gated.append(gd)
                lp = pslc_p.tile([P, C], F32, tag="lcps", name="lcps")
                for n in range(2):
                    sl = slice(n * 512, (n + 1) * 512)
                    nc.tensor.matmul(lp[:, sl], ut[:], gated[n][:],
                                     start=True, stop=True)
                nc.scalar.activation(lc_sb[j][:], lp[:], ACTF.Copy)
                nc.sync.dma_start(cs_sb[j:j + 1, :], lc_sb[j][127:128, :])

        # ================ carries + collective ================
        with tc.tile_pool(name="car", bufs=1) as car_p, \
             tc.tile_pool(name="pscar", bufs=1, space="PSUM") as pscar_p:
            tot_ps = pscar_p.tile([1, C], F32, tag="tot")
            carx_ps = pscar_p.tile([NCH, C], F32, tag="carx")
            for n in range(2):
                sl = slice(n * 512, (n + 1) * 512)
                nc.tensor.matmul(tot_ps[:, sl], ones16_1[:], cs_sb[:, sl],
                                 start=True, stop=True)
            ccin_sb = car_p.tile([1, C], F32)
            nc.vector.tensor_scalar(ccin_sb[:], tot_ps[:], segm[:], None,
                                    op0=ALU.mult)
            nc.sync.dma_start(cc_in.ap(), ccin_sb[:])
            nc.gpsimd.collective_compute(
                "AllReduce", ALU.add, replica_groups=groups,
                ins=[cc_in.ap()], outs=[cc_out.ap()])
            # local prefix part runs while the collective is in flight
            for n in range(2):
                sl = slice(n * 512, (n + 1) * 512)
                nc.tensor.matmul(carx_ps[:, sl], l0[:], cs_sb[:, sl],
                                 start=True, stop=False)
            base_sb = car_p.tile([1, C], F32)
            nc.sync.dma_start(base_sb[:], cc_out.ap())
            basem = car_p.tile([1, C], BF16)
            nc.vector.tensor_scalar(basem[:], base_sb[:], usem[:], None,
                                    op0=ALU.mult)
            for n in range(2):
                sl = slice(n * 512, (n + 1) * 512)
                nc.tensor.matmul(carx_ps[:, sl], ones1_16[:],
                                 basem[:, sl], start=False, stop=True)

            # ncarry = LN(carries) over d segments
            c3 = carx_ps[:].rearrange("p (h d) -> p h d", d=D)
            r1 = car_p.tile([NCH, H], F32)
            nc.vector.tensor_reduce(r1[:], c3, axis=AX.X, op=ALU.add)
            sqc = car_p.tile([NCH, C], F32)
            nc.scalar.square(sqc[:], carx_ps[:])
            r2 = car_p.tile([NCH, H], F32)
            nc.vector.tensor_reduce(r2[:], sqc[:].rearrange(
                "p (h d) -> p h d", d=D), axis=AX.X, op=ALU.add)
            mu = car_p.tile([NCH, H], F32)
            nc.vector.tensor_scalar(mu[:], r1[:], 1.0 / D, None, op0=ALU.mult)
            em2 = car_p.tile([NCH, H], F32)
            nc.vector.tensor_scalar(em2[:], r2[:], 1.0 / D, None,
                                    op0=ALU.mult)
            musq = car_p.tile([NCH, H], F32)
            nc.vector.tensor_tensor(musq[:], mu[:], mu[:], op=ALU.mult)
            var = car_p.tile([NCH, H], F32)
            nc.vector.tensor_tensor(var[:], em2[:], musq[:], op=ALU.subtract)
            sd = car_p.tile([NCH, H], F32)
            nc.scalar.activation(sd[:], var[:], ACTF.Sqrt, bias=eps16[:])
            rstd = car_p.tile([NCH, H], F32)
            nc.vector.reciprocal(rstd[:], sd[:])
            mu_b = mu[:].unsqueeze(2).to_broadcast([NCH, H, D])
            rstd_b = rstd[:].unsqueeze(2).to_broadcast([NCH, H, D])
            cen = car_p.tile([NCH, C], F32)
            nc.vector.tensor_tensor(cen[:].rearrange("p (h d) -> p h d", d=D),
                                    c3, mu_b, op=ALU.subtract)
            if has_carry_gb:
                nrm = car_p.tile([NCH, C], F32)
                nc.vector.tensor_tensor(
                    nrm[:].rearrange("p (h d) -> p h d", d=D),
                    cen[:].rearrange("p (h d) -> p h d", d=D), rstd_b,
                    op=ALU.mult)
                cg_b = cgr[:].unsqueeze(1).to_broadcast([NCH, H, D])
                cb_b = cbr[:].unsqueeze(1).to_broadcast([NCH, H, D])
                nrm2 = car_p.tile([NCH, C], F32)
                nc.vector.tensor_tensor(
                    nrm2[:].rearrange("p (h d) -> p h d", d=D),
                    nrm[:].rearrange("p (h d) -> p h d", d=D), cg_b,
                    op=ALU.mult)
                nc.vector.tensor_tensor(
                    ncarry[:].rearrange("p (h d) -> p h d", d=D),
                    nrm2[:].rearrange("p (h d) -> p h d", d=D), cb_b,
                    op=ALU.add)
            else:
                nc.vector.tensor_tensor(
                    ncarry[:].rearrange("p (h d) -> p h d", d=D),
                    cen[:].rearrange("p (h d) -> p h d", d=D), rstd_b,
                    op=ALU.mult)

        # ===== phases C/D/E, software-pipelined per position group =====
        with ExitStack() as late:
            ctp = late.enter_context(tc.tile_pool(name="cardsT", bufs=2))
            otp = late.enter_context(tc.tile_pool(name="outT", bufs=2))
            pc_p = late.enter_context(tc.tile_pool(name="phC", bufs=2))
            pd_p = late.enter_context(tc.tile_pool(name="phD", bufs=2))
            pe_p = late.enter_context(tc.tile_pool(name="phE", bufs=2))
            pscl_p = late.enter_context(
                tc.tile_pool(name="pscl", bufs=2, space="PSUM"))
            psh1_p = late.enter_context(
                tc.tile_pool(name="psh1", bufs=2, space="PSUM"))
            psy_p = late.enter_context(
                tc.tile_pool(name="psy", bufs=2, space="PSUM"))

            def make_ct(pg):
                # transposed cards, blocked layout: block (jj, n, gg) holds
                # chans (4n+gg)*128..+128 on partitions, tokens of chunk
                # pg*4+jj on cols jj*1024 + n*512 + gg*128 ..+128
                return ctp.tile([P, 4 * C], BF16, tag="ctbig",
                                name=f"ctbig{pg}")

            def C_unit(pg, jj, ctbig):
                    j = pg * 4 + jj
                    cl = []
                    for n in range(2):
                        sl = slice(n * 512, (n + 1) * 512)
                        cp = pscl_p.tile([P, 512], F32, tag=f"cl{n}",
                                         name=f"cl{n}")
                        nc.tensor.matmul(cp[:], st[:], lc_sb[j][:, sl],
                                         start=True, stop=False)
                        cl.append(cp)
                    for n in range(2):
                        sl = slice(n * 512, (n + 1) * 512)
                        nc.tensor.matmul(cl[n][:],
                                         selb[:, j * P:(j + 1) * P],
                                         ncarry[:, sl],
                                         start=False, stop=True)
                    cards = pc_p.tile([P, C], BF16, tag="cards",
                                      name=f"cards{j}")
                    for n in range(2):
                        cln = cl[n]
                        cl3 = cln[:].rearrange("p (h d) -> p h d", d=D)
                        sq = pc_p.tile([P, 512], F32, tag=f"sq{n}",
                                       name=f"sq{n}", bufs=1)
                        nc.scalar.square(sq[:], cln[:])
                        r1c = pc_p.tile([P, HH], F32, tag=f"r1c{n}",
                                        name=f"r1c{n}")
                        nc.vector.tensor_reduce(r1c[:], cl3, axis=AX.X,
                                                op=ALU.add)
                        r2c = pc_p.tile([P, HH], F32, tag=f"r2c{n}",
                                        name=f"r2c{n}")
                        nc.vector.tensor_reduce(
                            r2c[:], sq[:].rearrange("p (h d) -> p h d", d=D),
                            axis=AX.X, op=ALU.add)
                        muc = pc_p.tile([P, HH], F32, tag=f"muc{n}",
                                        name=f"muc{n}")
                        nc.vector.tensor_scalar(muc[:], r1c[:], 1.0 / D,
                                                None, op0=ALU.mult)
                        em2c = pc_p.tile([P, HH], F32, tag=f"em2c{n}",
                                         name=f"em2c{n}")
                        nc.vector.tensor_scalar(em2c[:], r2c[:], 1.0 / D,
                                                None, op0=ALU.mult)
                        musqc = pc_p.tile([P, HH], F32, tag=f"musqc{n}",
                                          name=f"musqc{n}")
                        nc.vector.tensor_tensor(musqc[:], muc[:], muc[:],
                                                op=ALU.mult)
                        varc = pc_p.tile([P, HH], F32, tag=f"varc{n}",
                                         name=f"varc{n}")
                        nc.vector.tensor_tensor(varc[:], em2c[:], musqc[:],
                                                op=ALU.subtract)
                        sdc = pc_p.tile([P, HH], F32, tag=f"sdc{n}",
                                        name=f"sdc{n}")
                        nc.scalar.activation(sdc[:], varc[:], ACTF.Sqrt,
                                             bias=eps128[:])
                        rstdc = pc_p.tile([P, HH], F32, tag=f"rstdc{n}",
                                          name=f"rstdc{n}")
                        nc.vector.reciprocal(rstdc[:], sdc[:])
                        # cards = cl*rstd - mu*rstd (one V pass + one G pass)
                        ms = pc_p.tile([P, HH], BF16, tag=f"ms{n}",
                                       name=f"ms{n}")
                        nc.vector.tensor_tensor(ms[:], muc[:], rstdc[:],
                                                op=ALU.mult)
                        rstd_bc = rstdc[:].unsqueeze(2).to_broadcast(
                            [P, HH, D])
                        ms_bc = ms[:].unsqueeze(2).to_broadcast([P, HH, D])
                        ctmp = pc_p.tile([P, 512], BF16, tag=f"cenc{n}",
                                         name=f"cenc{n}", bufs=1)
                        nc.vector.tensor_tensor(
                            ctmp[:].rearrange("p (h d) -> p h d", d=D),
                            cl3, rstd_bc, op=ALU.mult)
                        sl = slice(n * 512, (n + 1) * 512)
                        nc.gpsimd.tensor_tensor(
                            cards[:, sl].rearrange("p (h d) -> p h d", d=D),
                            ctmp[:].rearrange("p (h d) -> p h d", d=D),
                            ms_bc, op=ALU.subtract)
                        base = jj * C + n * 512
                        out3 = ctbig[:, base:base + 512].rearrange(
                            "p (b c) -> p b c", c=P)
                        nc.sync.dma_start_transpose(
                            out3, cards[:, n * 512:(n + 1) * 512])

            def ct_accessor(ctbig):
                def cardsT_fn(g, o):
                    n, gg = g // 4, g % 4
                    col = n * 512 + gg * P
                    return ctbig[o:o + D, :].rearrange(
                        "p (jj q) -> p jj q", q=C)[:, :, col:col + P]
                return cardsT_fn

            def make_ot(pg):
                return [otp.tile([P, 512], BF16, tag=f"ot{g}",
                                 name=f"ot{pg}_{g}") for g in range(NG)]

            # half-pg head groups: 8 heads sharing one stationary offset
            HALVES = [[0 + 2 * i for i in range(8)],
                      [1 + 2 * i for i in range(8)]]

            def D_h1(pg, hi, cardsT):
                heads = HALVES[hi]
                o = (hi % 2) * D
                hbs = {}
                # long h1 bursts: one LDWEIGHTS per 2 matmuls
                for quad in (heads[i:i + 2] for i in range(0, 8, 2)):
                    hps = {}
                    for h in quad:
                        hps[h] = psh1_p.tile([P, 512], F32, tag="h1",
                                             name=f"h1_{pg}_{h}")
                    for h in quad:
                        g = h // 2
                        xsl = slice((pg % 2) * 512, (pg % 2) * 512 + 512)
                        nc.tensor.matmul(
                            hps[h][:], w1x[o:o + D, :],
                            xth[g][pg // 2][o:o + D, xsl],
                            start=True, stop=False)
                    for h in quad:
                        g = h // 2
                        nc.tensor.matmul(hps[h][:], w1c[o:o + D, :],
                                         cardsT(g, o), start=False,
                                         stop=True)
                    # evacuate h1 fast: hb = h1 + b1 (bf16 out)
                    for h in quad:
                        hb = pd_p.tile([P, 512], BF16, tag=f"hb{h // 2}",
                                       name=f"hb_{pg}_{h}", bufs=1)
                        if h % 4 // 2 == 0:
                            nc.vector.tensor_scalar(hb[:], hps[h][:],
                                                    b1c[:], None,
                                                    op0=ALU.add)
                        else:
                            nc.scalar.activation(hb[:], hps[h][:],
                                                 ACTF.Identity,
                                                 bias=b1c[:])
                        hbs[h] = hb
                sqs, e3s, us = {}, {}, {}
                for h in heads:
                    sq3 = pd_p.tile([P, 512], BF16, tag=f"sq3{h // 2}",
                                    name=f"sq3_{h}", bufs=2)
                    nc.vector.tensor_tensor(sq3[:], hbs[h][:],
                                            hbs[h][:], op=ALU.mult)
                    sqs[h] = sq3
                for h in heads:
                    e3 = pd_p.tile([P, 512], BF16, tag=f"e3{h // 2}",
                                   name=f"e3_{h}", bufs=2)
                    nc.scalar.activation(e3[:], sqs[h][:], ACTF.Exp,
                                         scale=-0.5)
                    e3s[h] = e3
                for h in heads:
                    u = pd_p.tile([P, 512], BF16, tag=f"u{h // 2}",
                                  name=f"u_{h}", bufs=1)
                    nc.vector.tensor_tensor(u[:], hbs[h][:], e3s[h][:],
                                            op=ALU.mult)
                    us[h] = u
                return hbs, us

            def D_o2(pg, hi, hbs, us, outT):
                heads = HALVES[hi]
                o = (hi % 2) * D
                for quad in (heads[i:i + 2] for i in range(0, 8, 2)):
                    ops = {}
                    for h in quad:
                        ops[h] = psh1_p.tile([P, 512], F32, tag="h1",
                                             name=f"o2_{h}")
                    for h in quad:
                        nc.tensor.matmul(ops[h][0:D, :], w2[:], hbs[h][:],
                                         start=True, stop=False)
                    for h in quad:
                        nc.tensor.matmul(ops[h][0:D, :], w2a[:], us[h][:],
                                         start=False, stop=True)
                    for i, h in enumerate(quad):
                        g = h // 2
                        if i % 2 == 0:
                            nc.vector.tensor_copy(outT[g][o:o + D, :],
                                                  ops[h][0:D, :])
                        else:
                            nc.scalar.copy(outT[g][o:o + D, :],
                                           ops[h][0:D, :])

            def E_tt(pg, tt, outT):
                    t_i = pg * 4 + tt
                    col = tt * P
                    xa = pe_p.tile([P, C], F32, tag="xa", name=f"xa{t_i}",
                                   bufs=1)
                    nc.sync.dma_start(xa[:],
                                      xn_d.ap()[t_i * P:(t_i + 1) * P, :])
                    yp = []
                    for n in range(2):
                        yp.append(psy_p.tile([P, 512], F32, tag="yps",
                                             name=f"yps{t_i}_{n}"))
                    for k in range(NG):
                        lhs = outT[k][:, col:col + P]
                        st_ = (k == 0)
                        sp = (k == NG - 1) and not has_pjb
                        for n in range(2):
                            sl = slice(n * 512, (n + 1) * 512)
                            nc.tensor.matmul(yp[n][:], lhs, pjw[k][:, sl],
                                             start=st_, stop=sp)
                    if has_pjb:
                        for n in range(2):
                            sl = slice(n * 512, (n + 1) * 512)
                            nc.tensor.matmul(yp[n][:], ones1r[:],
                                             pjb[:, sl],
                                             start=False, stop=True)
                    yraw, s1h, s2h = [], [], []
                    for n in range(2):
                        yr = pe_p.tile([P, 512], F32, tag=f"yraw{n}",
                                       name=f"yraw{t_i}_{n}")
                        s1n = pe_p.tile([P, 1], F32, tag=f"s1{n}",
                                        name=f"s1_{t_i}_{n}")
                        nc.scalar.activation(yr[:], yp[n][:], ACTF.Copy,
                                             accum_out=s1n[:])
                        yraw.append(yr)
                        s1h.append(s1n)
                    for n in range(2):
                        sc4 = pe_p.tile([P, 512], F32, tag="sc4",
                                        name=f"sc4_{t_i}_{n}", bufs=1)
                        s2n = pe_p.tile([P, 1], F32, tag=f"s2{n}",
                                        name=f"s2_{t_i}_{n}")
                        nc.scalar.activation(sc4[:], yraw[n][:], ACTF.Square,
                                             scale=1.0 / 32.0,
                                             accum_out=s2n[:])
                        s2h.append(s2n)
                    s1t = pe_p.tile([P, 1], F32, tag="s1t", name=f"s1t{t_i}")
                    nc.vector.tensor_tensor(s1t[:], s1h[0][:], s1h[1][:],
                                            op=ALU.add)
                    m1 = pe_p.tile([P, 1], F32, tag="m1", name=f"m1_{t_i}")
                    nc.vector.tensor_scalar(m1[:], s1t[:], 1.0 / C, None,
                                            op0=ALU.mult)
                    s2t = pe_p.tile([P, 1], F32, tag="s2t", name=f"s2t{t_i}")
                    nc.vector.tensor_tensor(s2t[:], s2h[0][:], s2h[1][:],
                                            op=ALU.add)
                    msq = pe_p.tile([P, 1], F32, tag="msq", name=f"msq{t_i}")
                    nc.vector.tensor_tensor(msq[:], m1[:], m1[:],
                                            op=ALU.mult)
                    var4 = pe_p.tile([P, 1], F32, tag="var4",
                                     name=f"var4_{t_i}")
                    nc.vector.tensor_tensor(var4[:], s2t[:], msq[:],
                                            op=ALU.subtract)
                    sd4 = pe_p.tile([P, 1], F32, tag="sd4",
                                    name=f"sd4_{t_i}")
                    nc.scalar.activation(sd4[:], var4[:], ACTF.Sqrt,
                                         bias=eps128[:])
                    rstd4 = pe_p.tile([P, 1], F32, tag="rstd4",
                                      name=f"rstd4_{t_i}")
                    nc.vector.reciprocal(rstd4[:], sd4[:])
                    yout = pe_p.tile([P, C], F32, tag="yout",
                                     name=f"yout{t_i}")
                    for n in range(2):
                        sl = slice(n * 512, (n + 1) * 512)
                        tn = pe_p.tile([P, 512], F32, tag=f"tn{n}",
                                       name=f"tn{t_i}_{n}", bufs=1)
                        nc.vector.tensor_scalar(tn[:], yraw[n][:], m1[:],
                                                rstd4[:], op0=ALU.subtract,
                                                op1=ALU.mult)
                        if has_ln_g:
                            nc.vector.tensor_tensor(tn[:], tn[:], lgr[:, sl],
                                                    op=ALU.mult)
                        if has_ln_b:
                            nc.vector.tensor_tensor(tn[:], tn[:], lbr[:, sl],
                                                    op=ALU.add)
                        if pg == NPG - 1:
                            # drain tail: vector is idle, gpsimd is slow
                            nc.vector.tensor_tensor(yout[:, sl], tn[:],
                                                    xa[:, sl], op=ALU.add)
                        else:
                            nc.gpsimd.tensor_tensor(yout[:, sl], tn[:],
                                                    xa[:, sl], op=ALU.add)
                    nc.sync.dma_start(y_d.ap()[t_i * P:(t_i + 1) * P, :],
                                      yout[:])

            # software pipeline, distance 2: C(pg) | D(pg-1) | E(pg-2).
            # Static interleave: E's dense matmul bursts fill the PE queue
            # while D's activation chain and C's LN chain are in flight.
            cts, ots = {}, {}
            for pg in range(NPG + 2):
                if pg < NPG:
                    cts[pg] = make_ct(pg)
                if 1 <= pg <= NPG:
                    ots[pg - 1] = make_ot(pg - 1)

                def d_h1(hi):
                    if 1 <= pg <= NPG:
                        return D_h1(pg - 1, hi, ct_accessor(cts[pg - 1]))
                    return None

                def d_o2(hi, st_):
                    if st_ is not None:
                        D_o2(pg - 1, hi, st_[0], st_[1], ots[pg - 1])

                def c_unit(i):
                    if pg < NPG:
                        C_unit(pg, i, cts[pg])

                def e_tt(i):
                    if pg >= 2:
                        E_tt(pg - 2, i, ots[pg - 2])

                stA = d_h1(0)
                e_tt(0)
                c_unit(0)
                d_o2(0, stA)
                e_tt(1)
                c_unit(1)
                stB = d_h1(1)
                e_tt(2)
                c_unit(2)
                d_o2(1, stB)
                e_tt(3)
                c_unit(3)
                if pg >= 2:
                    del cts[pg - 2], ots[pg - 2]

    nc.compile()
    return nc


_CACHE = {}


def _get_program(alpha, flags):
    key = (alpha, flags)
    if key not in _CACHE:
        _CACHE[key] = _build(NCORES, alpha, *flags)
    return _CACHE[key]


def _bf16(a):
    return np.ascontiguousarray(a.astype(ml_dtypes.bfloat16))


def prepare(inputs):
    """Compute flags + the per-core input maps (host-side prep)."""
    x = np.ascontiguousarray(np.asarray(inputs["x"], np.float32))
    mark_W = np.asarray(inputs["mark_W"], np.float32)
    mark_b = np.asarray(inputs["mark_b"], np.float32)
    gate_W = np.asarray(inputs["gate_W"], np.float32)
    gate_b = np.asarray(inputs["gate_b"], np.float32)
    carry_g = np.asarray(inputs["carry_g"], np.float32)
    carry_b = np.asarray(inputs["carry_b"], np.float32)
    card_g = np.asarray(inputs["card_g"], np.float32)
    card_b = np.asarray(inputs["card_b"], np.float32)
    W1 = np.asarray(inputs["W1"], np.float32)
    b1 = np.asarray(inputs["b1"], np.float32)
    alpha = float(np.asarray(inputs["alpha"]))
    W2 = np.asarray(inputs["W2"], np.float32)
    b2 = np.asarray(inputs["b2"], np.float32)
    proj_W = np.asarray(inputs["proj_W"], np.float32)
    proj_b = np.asarray(inputs["proj_b"], np.float32)
    ln_g = np.asarray(inputs["ln_g"], np.float32)
    ln_b = np.asarray(inputs["ln_b"], np.float32)

    W1x = np.concatenate([W1[:D, :], W1[:D, :]], 0)
    W1c0 = card_g[:, None] * W1[D:, :]
    W1c = np.concatenate([W1c0, W1c0], 0)
    b1f = (b1 + card_b @ W1[D:, :]).astype(np.float32)
    # fold b2 into the proj bias: y = ho @ proj_W + proj_b, ho = .. + b2
    row = np.tile(b2, H).astype(np.float32)
    pjb_eff = (proj_b + row @ proj_W).astype(np.float32)

    stm = np.zeros((P, P), np.float32)
    for i in range(1, P):
        stm[i - 1, i] = 1.0

    has_carry_gb = bool(np.any(carry_g != 1.0) or np.any(carry_b != 0.0))
    flags = (bool(np.any(mark_b)), bool(np.any(gate_b)),
             bool(np.any(pjb_eff)), has_carry_gb,
             bool(np.any(ln_g != 1.0)), bool(np.any(ln_b)))

    common = {
        "mkw": _bf16(mark_W), "gtw": _bf16(gate_W), "pjw": _bf16(proj_W),
        "mkb": _bf16(mark_b[None, :]), "gtb": _bf16(gate_b[None, :]),
        "pjb": _bf16(pjb_eff[None, :]),
        "w1x": _bf16(W1x), "w1c": _bf16(W1c), "b1c": b1f[:, None],
        "w2": _bf16(W2), "w2a": _bf16(alpha * W2),
        "ut": _bf16(np.triu(np.ones((P, P), np.float32))),
        "st": np.ascontiguousarray(stm.astype(ml_dtypes.bfloat16)),
        "l0": _bf16(np.triu(np.ones((NCH, NCH), np.float32), k=1)),
        "onesr": _bf16(np.ones((1, P), np.float32)),
        "selb": _bf16(np.concatenate(
            [np.eye(NCH, dtype=np.float32)[:, j:j + 1] * np.ones((1, P))
             for j in range(NCH)], axis=1)),
        "cgr": np.tile(carry_g[None, :], (NCH, 1)).astype(np.float32),
        "cbr": np.tile(carry_b[None, :], (NCH, 1)).astype(np.float32),
        "lgr": np.tile(ln_g[None, :], (P, 1)).astype(np.float32),
        "lbr": np.tile(ln_b[None, :], (P, 1)).astype(np.float32),
    }
    in_maps = []
    for c in range(NCORES):
        b, half = c // 2, c % 2
        xs = x[b, half * R:(half + 1) * R, :]
        m = dict(common)
        m["xn"] = np.ascontiguousarray(xs)
        m["xt"] = _bf16(xs.T)
        m["segm"] = np.array([[1.0 - half]], np.float32)
        m["usem"] = np.array([[float(half)]], np.float32)
        in_maps.append(m)
    return alpha, flags, in_maps


def kernel(**inputs):
    alpha, flags, in_maps = prepare(inputs)
    nc = _get_program(alpha, flags)
    res = run_bass_kernel_spmd(nc, in_maps, list(range(NCORES)))
    out = np.empty((B, T, C), np.float32)
    for c in range(NCORES):
        b, half = c // 2, c % 2
        out[b, half * R:(half + 1) * R, :] = res.results[c]["y"]
    return out


# revision 88
# speedup vs baseline: 1.0207x; 1.0207x over previous
"""Trainium2 Bass kernel for nn_ChunkedMultiHeadCardPassingLayer.

Sharding: 8 cores = (batch b = core//2) x (T-half = core%2). Each core
processes 2048 contiguous tokens of one batch end-to-end; the only
cross-core dependency is the chunk-carry prefix, resolved with a 4KB
paired AllReduce.

Restructure vs the original baseline (596us -> ~495us):
- all matmul operands are 2-byte (bf16) -> cheap LDWEIGHTS, less DMA
- x kept transposed + resident in SBUF for phases A and D
- local_cum kept in SBUF as bf16 (no DRAM spill round-trip)
- chunk sums extracted from cumsum row 127 via tiny DMA (csel dropped)
- ncarry broadcast via selector-stationary matmul (no bounce DMAs)
- cards transposed via blocked XBAR DMA-transpose, 2 calls per chunk
  (no PE transposes, no PSUM pressure, no evac copies)
- MLP tail uses matmul linearity: o2 = W2.T@hb + (alpha*W2).T@(hb*e3),
  removing an elementwise pass; b1/b2 folded into downstream biases
- activation chain spread across scalar/vector/gpsimd; same-function
  scalar ops batched to limit ACT table reloads
- software pipeline at distance 2 (C(pg) | D(pg-1) | E(pg-2)) with a
  static unit interleave so E's dense matmul bursts keep the PE busy
  while D's activation chain and C's LN chain are in flight
"""
import os
os.environ.setdefault("JAX_PLATFORMS", "cpu")

import numpy as np
import ml_dtypes
from contextlib import ExitStack

import concourse.bacc as bacc
import concourse.mybir as mybir
import concourse.tile as tile
from concourse.bass_utils import run_bass_kernel_spmd

F32 = mybir.dt.float32
F32R = mybir.dt.float32r
BF16 = mybir.dt.bfloat16
AX = mybir.AxisListType
ALU = mybir.AluOpType
ACTF = mybir.ActivationFunctionType

# problem constants
B, T, C = 4, 4096, 1024
H, CS = 16, 128
D = C // H            # 64
NCORES = 8
R = T // 2            # 2048 rows per core
NCH = R // CS         # 16 chunks per core
NG = C // 128         # 8 groups of (2 heads x 64)
NPG = NCH // 4        # 4 position groups of 512
EPS = 1e-5
P = 128
HH = 8                # heads per 512 half


def _build(ncores, alpha, has_mkb, has_gtb, has_pjb,
           has_carry_gb, has_ln_g, has_ln_b):
    nc = bacc.Bacc("TRN2", target_bir_lowering=False, debug=False,
                   num_devices=ncores)

    # ---------------- DRAM I/O ----------------
    xt_d = nc.dram_tensor("xt", [C, R], BF16, kind="ExternalInput")
    xn_d = nc.dram_tensor("xn", [R, C], F32, kind="ExternalInput")
    mkw_d = nc.dram_tensor("mkw", [C, C], BF16, kind="ExternalInput")
    gtw_d = nc.dram_tensor("gtw", [C, C], BF16, kind="ExternalInput")
    pjw_d = nc.dram_tensor("pjw", [C, C], BF16, kind="ExternalInput")
    mkb_d = nc.dram_tensor("mkb", [1, C], BF16, kind="ExternalInput")
    gtb_d = nc.dram_tensor("gtb", [1, C], BF16, kind="ExternalInput")
    pjb_d = nc.dram_tensor("pjb", [1, C], BF16, kind="ExternalInput")
    w1x_d = nc.dram_tensor("w1x", [2 * D, 2 * D], BF16, kind="ExternalInput")
    w1c_d = nc.dram_tensor("w1c", [2 * D, 2 * D], BF16, kind="ExternalInput")
    b1_d = nc.dram_tensor("b1c", [2 * D, 1], F32, kind="ExternalInput")
    w2_d = nc.dram_tensor("w2", [2 * D, D], BF16, kind="ExternalInput")
    w2a_d = nc.dram_tensor("w2a", [2 * D, D], BF16, kind="ExternalInput")
    ut_d = nc.dram_tensor("ut", [P, P], BF16, kind="ExternalInput")
    st_d = nc.dram_tensor("st", [P, P], BF16, kind="ExternalInput")
    l0_d = nc.dram_tensor("l0", [NCH, NCH], BF16, kind="ExternalInput")
    onesr_d = nc.dram_tensor("onesr", [1, P], BF16, kind="ExternalInput")
    selb_d = nc.dram_tensor("selb", [NCH, NCH * P], BF16,
                            kind="ExternalInput")
    segm_d = nc.dram_tensor("segm", [1, 1], F32, kind="ExternalInput")
    usem_d = nc.dram_tensor("usem", [1, 1], F32, kind="ExternalInput")
    cgr_d = nc.dram_tensor("cgr", [NCH, D], F32, kind="ExternalInput")
    cbr_d = nc.dram_tensor("cbr", [NCH, D], F32, kind="ExternalInput")
    lgr_d = nc.dram_tensor("lgr", [P, C], F32, kind="ExternalInput")
    lbr_d = nc.dram_tensor("lbr", [P, C], F32, kind="ExternalInput")

    y_d = nc.dram_tensor("y", [R, C], F32, kind="ExternalOutput")

    cc_in = nc.dram_tensor("cc_in", [1, C], F32)
    cc_out = nc.dram_tensor("cc_out", [1, C], F32)

    groups = ([[i, i + 1] for i in range(0, ncores, 2)]
              if ncores > 1 else [[0]])

    with tile.TileContext(nc) as tc, ExitStack() as top:
        const_p = top.enter_context(tc.tile_pool(name="const", bufs=1))
        xt_p = top.enter_context(tc.tile_pool(name="xtp", bufs=1))
        lc_p = top.enter_context(tc.tile_pool(name="lcp", bufs=1))
        carr_p = top.enter_context(tc.tile_pool(name="carr", bufs=1))

        # ---------- constants ----------
        ut = const_p.tile([P, P], BF16)
        st = const_p.tile([P, P], BF16)
        l0 = const_p.tile([NCH, NCH], BF16)
        w1x = const_p.tile([2 * D, 2 * D], BF16)
        w1c = const_p.tile([2 * D, 2 * D], BF16)
        b1c = const_p.tile([2 * D, 1], F32)
        w2 = const_p.tile([2 * D, D], BF16)
        w2a = const_p.tile([2 * D, D], BF16)
        segm = const_p.tile([1, 1], F32)
        usem = const_p.tile([1, 1], F32)
        ones1r = const_p.tile([1, P], BF16)
        selb = const_p.tile([NCH, NCH * P], BF16)
        for t_, d_ in ((ut, ut_d), (st, st_d), (l0, l0_d),
                       (w1x, w1x_d), (w1c, w1c_d), (b1c, b1_d),
                       (w2, w2_d), (w2a, w2a_d), (segm, segm_d),
                       (usem, usem_d), (ones1r, onesr_d), (selb, selb_d)):
            nc.sync.dma_start(t_[:], d_.ap())
        ones16_1 = const_p.tile([NCH, 1], BF16)
        nc.vector.memset(ones16_1[:], 1.0)
        ones1_16 = const_p.tile([1, NCH], BF16)
        nc.vector.memset(ones1_16[:], 1.0)
        eps128 = const_p.tile([P, 1], F32)
        nc.vector.memset(eps128[:], EPS)
        eps16 = const_p.tile([NCH, 1], F32)
        nc.vector.memset(eps16[:], EPS)
        if has_mkb or has_gtb:
            mkb = const_p.tile([1, C], BF16)
            gtb = const_p.tile([1, C], BF16)
            nc.sync.dma_start(mkb[:], mkb_d.ap())
            nc.sync.dma_start(gtb[:], gtb_d.ap())
        if has_pjb:
            pjb = const_p.tile([1, C], BF16)
            nc.sync.dma_start(pjb[:], pjb_d.ap())
        if has_carry_gb:
            cgr = const_p.tile([NCH, D], F32)
            cbr = const_p.tile([NCH, D], F32)
            nc.sync.dma_start(cgr[:], cgr_d.ap())
            nc.sync.dma_start(cbr[:], cbr_d.ap())

        # resident x (transposed), one tile per chan-group
        xt = [xt_p.tile([P, R], BF16, tag=f"xt{g}", name=f"xt{g}")
              for g in range(NG)]
        # resident pjw (loaded later; pool allocated at top level)
        pjw_p = top.enter_context(tc.tile_pool(name="pjp", bufs=1))
        pjw = [pjw_p.tile([P, C], BF16, tag=f"pj{k}", name=f"pj{k}")
               for k in range(NG)]
        lgr = pjw_p.tile([P, C], F32) if has_ln_g else None
        lbr = pjw_p.tile([P, C], F32) if has_ln_b else None

        # resident local_cum (bf16) + chunk sums + normalized carries
        lc_sb = []
        for j in range(NCH):
            t_ = lc_p.tile([P, C], BF16, tag=f"lc{j}", name=f"lc{j}")
            lc_sb.append(t_)
        cs_sb = carr_p.tile([NCH, C], BF16)
        ncarry = carr_p.tile([NCH, C], BF16)

        # ================ phase A: pm/gate/scan ================
        with tc.tile_pool(name="wgt", bufs=1) as wgt_p, \
             tc.tile_pool(name="ph1", bufs=2) as ph1_p, \
             tc.tile_pool(name="psA", bufs=1, space="PSUM") as psA_p, \
             tc.tile_pool(name="pslc", bufs=2, space="PSUM") as pslc_p:
            mkw, gtw = [], []
            for k in range(NG):
                mt = wgt_p.tile([P, C], BF16, tag=f"mk{k}", name=f"mk{k}")
                gt_ = wgt_p.tile([P, C], BF16, tag=f"gk{k}", name=f"gk{k}")
                nc.sync.dma_start(mt[:], mkw_d.ap()[k * P:(k + 1) * P, :])
                nc.sync.dma_start(gt_[:], gtw_d.ap()[k * P:(k + 1) * P, :])
                mkw.append(mt)
                gtw.append(gt_)
            for g in range(NG):
                nc.sync.dma_start(xt[g][:], xt_d.ap()[g * P:(g + 1) * P, :])
            for k in range(NG):
                nc.sync.dma_start(pjw[k][:], pjw_d.ap()[k * P:(k + 1) * P, :])
            if has_ln_g:
                nc.sync.dma_start(lgr[:], lgr_d.ap())
            if has_ln_b:
                nc.sync.dma_start(lbr[:], lbr_d.ap())
            for j in range(NCH):
                pm0 = psA_p.tile([P, 512], F32, tag="pm0", name="pm0")
                gt0 = psA_p.tile([P, 512], F32, tag="gt0", name="gt0")
                pm1 = psA_p.tile([P, 512], F32, tag="pm1", name="pm1")
                gt1 = psA_p.tile([P, 512], F32, tag="gt1", name="gt1")
                s0, s1_ = slice(0, 512), slice(512, 1024)
                for k in range(NG):
                    lhs = xt[k][:, j * P:(j + 1) * P]
                    st_ = (k == 0)
                    spm = (k == NG - 1) and not has_mkb
                    spg = (k == NG - 1) and not has_gtb
                    nc.tensor.matmul(pm0[:], lhs, mkw[k][:, s0],
                                     start=st_, stop=spm)
                    nc.tensor.matmul(gt0[:], lhs, gtw[k][:, s0],
                                     start=st_, stop=spg)
                    nc.tensor.matmul(pm1[:], lhs, mkw[k][:, s1_],
                                     start=st_, stop=spm)
                    nc.tensor.matmul(gt1[:], lhs, gtw[k][:, s1_],
                                     start=st_, stop=spg)
                if has_mkb:
                    nc.tensor.matmul(pm0[:], ones1r[:], mkb[:, s0],
                                     start=False, stop=True)
                    nc.tensor.matmul(pm1[:], ones1r[:], mkb[:, s1_],
                                     start=False, stop=True)
                if has_gtb:
                    nc.tensor.matmul(gt0[:], ones1r[:], gtb[:, s0],
                                     start=False, stop=True)
                    nc.tensor.matmul(gt1[:], ones1r[:], gtb[:, s1_],
                                     start=False, stop=True)
                gated = []
                for n, (pm_ps, gt_ps) in enumerate(((pm0, gt0), (pm1, gt1))):
                    gates = ph1_p.tile([P, 512], F32, tag=f"gates{n}",
                                       name=f"gates{n}")
                    nc.scalar.activation(gates[:], gt_ps[:], ACTF.Sigmoid)
                    gd = ph1_p.tile([P, 512], BF16, tag=f"gated{n}",
                                    name=f"gated{n}")
                    nc.vector.tensor_tensor(gd[:], gates[:], pm_ps[:],
                                            op=ALU.mult)
                    gated.append(gd)
                lp = pslc_p.tile([P, C], F32, tag="lcps", name="lcps")
                for n in range(2):
                    sl = slice(n * 512, (n + 1) * 512)
                    nc.tensor.matmul(lp[:, sl], ut[:], gated[n][:],
                                     start=True, stop=True)
                nc.scalar.activation(lc_sb[j][:], lp[:], ACTF.Copy)
                nc.sync.dma_start(cs_sb[j:j + 1, :], lc_sb[j][127:128, :])

        # ================ carries + collective ================
        with tc.tile_pool(name="car", bufs=1) as car_p, \
             tc.tile_pool(name="pscar", bufs=1, space="PSUM") as pscar_p:
            tot_ps = pscar_p.tile([1, C], F32, tag="tot")
            carx_ps = pscar_p.tile([NCH, C], F32, tag="carx")
            for n in range(2):
                sl = slice(n * 512, (n + 1) * 512)
                nc.tensor.matmul(tot_ps[:, sl], ones16_1[:], cs_sb[:, sl],
                                 start=True, stop=True)
            ccin_sb = car_p.tile([1, C], F32)
            nc.vector.tensor_scalar(ccin_sb[:], tot_ps[:], segm[:], None,
                                    op0=ALU.mult)
            nc.sync.dma_start(cc_in.ap(), ccin_sb[:])
            nc.gpsimd.collective_compute(
                "AllReduce", ALU.add, replica_groups=groups,
                ins=[cc_in.ap()], outs=[cc_out.ap()])
            # local prefix part runs while the collective is in flight
            for n in range(2):
                sl = slice(n * 512, (n + 1) * 512)
                nc.tensor.matmul(carx_ps[:, sl], l0[:], cs_sb[:, sl],
                                 start=True, stop=False)
            base_sb = car_p.tile([1, C], F32)
            nc.sync.dma_start(base_sb[:], cc_out.ap())
            basem = car_p.tile([1, C], BF16)
            nc.vector.tensor_scalar(basem[:], base_sb[:], usem[:], None,
                                    op0=ALU.mult)
            for n in range(2):
                sl = slice(n * 512, (n + 1) * 512)
                nc.tensor.matmul(carx_ps[:, sl], ones1_16[:],
                                 basem[:, sl], start=False, stop=True)

            # ncarry = LN(carries) over d segments
            c3 = carx_ps[:].rearrange("p (h d) -> p h d", d=D)
            r1 = car_p.tile([NCH, H], F32)
            nc.vector.tensor_reduce(r1[:], c3, axis=AX.X, op=ALU.add)
            sqc = car_p.tile([NCH, C], F32)
            nc.scalar.square(sqc[:], carx_ps[:])
            r2 = car_p.tile([NCH, H], F32)
            nc.vector.tensor_reduce(r2[:], sqc[:].rearrange(
                "p (h d) -> p h d", d=D), axis=AX.X, op=ALU.add)
            mu = car_p.tile([NCH, H], F32)
            nc.vector.tensor_scalar(mu[:], r1[:], 1.0 / D, None, op0=ALU.mult)
            em2 = car_p.tile([NCH, H], F32)
            nc.vector.tensor_scalar(em2[:], r2[:], 1.0 / D, None,
                                    op0=ALU.mult)
            musq = car_p.tile([NCH, H], F32)
            nc.vector.tensor_tensor(musq[:], mu[:], mu[:], op=ALU.mult)
            var = car_p.tile([NCH, H], F32)
            nc.vector.tensor_tensor(var[:], em2[:], musq[:], op=ALU.subtract)
            sd = car_p.tile([NCH, H], F32)
            nc.scalar.activation(sd[:], var[:], ACTF.Sqrt, bias=eps16[:])
            rstd = car_p.tile([NCH, H], F32)
            nc.vector.reciprocal(rstd[:], sd[:])
            mu_b = mu[:].unsqueeze(2).to_broadcast([NCH, H, D])
            rstd_b = rstd[:].unsqueeze(2).to_broadcast([NCH, H, D])
            cen = car_p.tile([NCH, C], F32)
            nc.vector.tensor_tensor(cen[:].rearrange("p (h d) -> p h d", d=D),
                                    c3, mu_b, op=ALU.subtract)
            if has_carry_gb:
                nrm = car_p.tile([NCH, C], F32)
                nc.vector.tensor_tensor(
                    nrm[:].rearrange("p (h d) -> p h d", d=D),
                    cen[:].rearrange("p (h d) -> p h d", d=D), rstd_b,
                    op=ALU.mult)
                cg_b = cgr[:].unsqueeze(1).to_broadcast([NCH, H, D])
                cb_b = cbr[:].unsqueeze(1).to_broadcast([NCH, H, D])
                nrm2 = car_p.tile([NCH, C], F32)
                nc.vector.tensor_tensor(
                    nrm2[:].rearrange("p (h d) -> p h d", d=D),
                    nrm[:].rearrange("p (h d) -> p h d", d=D), cg_b,
                    op=ALU.mult)
                nc.vector.tensor_tensor(
                    ncarry[:].rearrange("p (h d) -> p h d", d=D),
                    nrm2[:].rearrange("p (h d) -> p h d", d=D), cb_b,
                    op=ALU.add)
            else:
                nc.vector.tensor_tensor(
                    ncarry[:].rearrange("p (h d) -> p h d", d=D),
                    cen[:].rearrange("p (h d) -> p h d", d=D), rstd_b,
                    op=ALU.mult)

        # ===== phases C/D/E, software-pipelined per position group =====
        with ExitStack() as late:
            ctp = late.enter_context(tc.tile_pool(name="cardsT", bufs=2))
            otp = late.enter_context(tc.tile_pool(name="outT", bufs=2))
            pc_p = late.enter_context(tc.tile_pool(name="phC", bufs=2))
            pd_p = late.enter_context(tc.tile_pool(name="phD", bufs=2))
            pe_p = late.enter_context(tc.tile_pool(name="phE", bufs=2))
            pscl_p = late.enter_context(
                tc.tile_pool(name="pscl", bufs=2, space="PSUM"))
            psh1_p = late.enter_context(
                tc.tile_pool(name="psh1", bufs=2, space="PSUM"))
            psy_p = late.enter_context(
                tc.tile_pool(name="psy", bufs=2, space="PSUM"))

            def make_ct(pg):
                # transposed cards, blocked layout: block (jj, n, gg) holds
                # chans (4n+gg)*128..+128 on partitions, tokens of chunk
                # pg*4+jj on cols jj*1024 + n*512 + gg*128 ..+128
                return ctp.tile([P, 4 * C], BF16, tag="ctbig",
                                name=f"ctbig{pg}")

            def C_unit(pg, jj, ctbig):
                    j = pg * 4 + jj
                    cl = []
                    for n in range(2):
                        sl = slice(n * 512, (n + 1) * 512)
                        cp = pscl_p.tile([P, 512], F32, tag=f"cl{n}",
                                         name=f"cl{n}")
                        nc.tensor.matmul(cp[:], st[:], lc_sb[j][:, sl],
                                         start=True, stop=False)
                        cl.append(cp)
                    for n in range(2):
                        sl = slice(n * 512, (n + 1) * 512)
                        nc.tensor.matmul(cl[n][:],
                                         selb[:, j * P:(j + 1) * P],
                                         ncarry[:, sl],
                                         start=False, stop=True)
                    cards = pc_p.tile([P, C], BF16, tag="cards",
                                      name=f"cards{j}")
                    for n in range(2):
                        cln = cl[n]
                        cl3 = cln[:].rearrange("p (h d) -> p h d", d=D)
                        sq = pc_p.tile([P, 512], F32, tag=f"sq{n}",
                                       name=f"sq{n}", bufs=1)
                        nc.scalar.square(sq[:], cln[:])
                        r1c = pc_p.tile([P, HH], F32, tag=f"r1c{n}",
                                        name=f"r1c{n}")
                        nc.vector.tensor_reduce(r1c[:], cl3, axis=AX.X,
                                                op=ALU.add)
                        r2c = pc_p.tile([P, HH], F32, tag=f"r2c{n}",
                                        name=f"r2c{n}")
                        nc.vector.tensor_reduce(
                            r2c[:], sq[:].rearrange("p (h d) -> p h d", d=D),
                            axis=AX.X, op=ALU.add)
                        muc = pc_p.tile([P, HH], F32, tag=f"muc{n}",
                                        name=f"muc{n}")
                        nc.vector.tensor_scalar(muc[:], r1c[:], 1.0 / D,
                                                None, op0=ALU.mult)
                        em2c = pc_p.tile([P, HH], F32, tag=f"em2c{n}",
                                         name=f"em2c{n}")
                        nc.vector.tensor_scalar(em2c[:], r2c[:], 1.0 / D,
                                                None, op0=ALU.mult)
                        musqc = pc_p.tile([P, HH], F32, tag=f"musqc{n}",
                                          name=f"musqc{n}")
                        nc.vector.tensor_tensor(musqc[:], muc[:], muc[:],
                                                op=ALU.mult)
                        varc = pc_p.tile([P, HH], F32, tag=f"varc{n}",
                                         name=f"varc{n}")
                        nc.vector.tensor_tensor(varc[:], em2c[:], musqc[:],
                                                op=ALU.subtract)
                        sdc = pc_p.tile([P, HH], F32, tag=f"sdc{n}",
                                        name=f"sdc{n}")
                        nc.scalar.activation(sdc[:], varc[:], ACTF.Sqrt,
                                             bias=eps128[:])
                        rstdc = pc_p.tile([P, HH], F32, tag=f"rstdc{n}",
                                          name=f"rstdc{n}")
                        nc.vector.reciprocal(rstdc[:], sdc[:])
                        # cards = cl*rstd - mu*rstd (one V pass + one G pass)
                        ms = pc_p.tile([P, HH], BF16, tag=f"ms{n}",
                                       name=f"ms{n}")
                        nc.vector.tensor_tensor(ms[:], muc[:], rstdc[:],
                                                op=ALU.mult)
                        rstd_bc = rstdc[:].unsqueeze(2).to_broadcast(
                            [P, HH, D])
                        ms_bc = ms[:].unsqueeze(2).to_broadcast([P, HH, D])
                        ctmp = pc_p.tile([P, 512], BF16, tag=f"cenc{n}",
                                         name=f"cenc{n}", bufs=1)
                        nc.vector.tensor_tensor(
                            ctmp[:].rearrange("p (h d) -> p h d", d=D),
                            cl3, rstd_bc, op=ALU.mult)
                        sl = slice(n * 512, (n + 1) * 512)
                        nc.gpsimd.tensor_tensor(
                            cards[:, sl].rearrange("p (h d) -> p h d", d=D),
                            ctmp[:].rearrange("p (h d) -> p h d", d=D),
                            ms_bc, op=ALU.subtract)
                    for n in range(2):
                        base = jj * C + n * 512
                        out3 = ctbig[:, base:base + 512].rearrange(
                            "p (b c) -> p b c", c=P)
                        nc.sync.dma_start_transpose(
                            out3, cards[:, n * 512:(n + 1) * 512])

            def ct_accessor(ctbig):
                def cardsT_fn(g, o):
                    n, gg = g // 4, g % 4
                    col = n * 512 + gg * P
                    return ctbig[o:o + D, :].rearrange(
                        "p (jj q) -> p jj q", q=C)[:, :, col:col + P]
                return cardsT_fn

            def make_ot(pg):
                return [otp.tile([P, 512], BF16, tag=f"ot{g}",
                                 name=f"ot{pg}_{g}") for g in range(NG)]

            # half-pg head groups: 8 heads sharing one stationary offset
            HALVES = [[0 + 2 * i for i in range(8)],
                      [1 + 2 * i for i in range(8)]]

            def D_h1(pg, hi, cardsT):
                heads = HALVES[hi]
                o = (hi % 2) * D
                hbs = {}
                # long h1 bursts: one LDWEIGHTS per 2 matmuls
                for quad in (heads[i:i + 2] for i in range(0, 8, 2)):
                    hps = {}
                    for h in quad:
                        hps[h] = psh1_p.tile([P, 512], F32, tag="h1",
                                             name=f"h1_{pg}_{h}")
                    for h in quad:
                        g = h // 2
                        nc.tensor.matmul(
                            hps[h][:], w1x[o:o + D, :],
                            xt[g][o:o + D, pg * 512:(pg + 1) * 512],
                            start=True, stop=False)
                    for h in quad:
                        g = h // 2
                        nc.tensor.matmul(hps[h][:], w1c[o:o + D, :],
                                         cardsT(g, o), start=False,
                                         stop=True)
                    # evacuate h1 fast: hb = h1 + b1 (bf16 out)
                    for h in quad:
                        hb = pd_p.tile([P, 512], BF16, tag=f"hb{h // 2}",
                                       name=f"hb_{pg}_{h}", bufs=1)
                        if h % 4 // 2 == 0:
                            nc.vector.tensor_scalar(hb[:], hps[h][:],
                                                    b1c[:], None,
                                                    op0=ALU.add)
                        else:
                            nc.scalar.activation(hb[:], hps[h][:],
                                                 ACTF.Identity,
                                                 bias=b1c[:])
                        hbs[h] = hb
                sqs, e3s, us = {}, {}, {}
                for h in heads:
                    sq3 = pd_p.tile([P, 512], BF16, tag=f"sq3{h // 2}",
                                    name=f"sq3_{h}", bufs=2)
                    nc.vector.tensor_tensor(sq3[:], hbs[h][:],
                                            hbs[h][:], op=ALU.mult)
                    sqs[h] = sq3
                for h in heads:
                    e3 = pd_p.tile([P, 512], BF16, tag=f"e3{h // 2}",
                                   name=f"e3_{h}", bufs=2)
                    nc.scalar.activation(e3[:], sqs[h][:], ACTF.Exp,
                                         scale=-0.5)
                    e3s[h] = e3
                for h in heads:
                    u = pd_p.tile([P, 512], BF16, tag=f"u{h // 2}",
                                  name=f"u_{h}", bufs=1)
                    nc.vector.tensor_tensor(u[:], hbs[h][:], e3s[h][:],
                                            op=ALU.mult)
                    us[h] = u
                return hbs, us

            def D_o2(pg, hi, hbs, us, outT):
                heads = HALVES[hi]
                o = (hi % 2) * D
                for quad in (heads[i:i + 2] for i in range(0, 8, 2)):
                    ops = {}
                    for h in quad:
                        ops[h] = psh1_p.tile([P, 512], F32, tag="h1",
                                             name=f"o2_{h}")
                    for h in quad:
                        nc.tensor.matmul(ops[h][0:D, :], w2[:], hbs[h][:],
                                         start=True, stop=False)
                    for h in quad:
                        nc.tensor.matmul(ops[h][0:D, :], w2a[:], us[h][:],
                                         start=False, stop=True)
                    for i, h in enumerate(quad):
                        g = h // 2
                        if i % 2 == 0:
                            nc.vector.tensor_copy(outT[g][o:o + D, :],
                                                  ops[h][0:D, :])
                        else:
                            nc.scalar.copy(outT[g][o:o + D, :],
                                           ops[h][0:D, :])

            def E_tt(pg, tt, outT):
                    t_i = pg * 4 + tt
                    col = tt * P
                    xa = pe_p.tile([P, C], F32, tag="xa", name=f"xa{t_i}",
                                   bufs=1)
                    nc.sync.dma_start(xa[:],
                                      xn_d.ap()[t_i * P:(t_i + 1) * P, :])
                    yp = []
                    for n in range(2):
                        yp.append(psy_p.tile([P, 512], F32, tag="yps",
                                             name=f"yps{t_i}_{n}"))
                    for k in range(NG):
                        lhs = outT[k][:, col:col + P]
                        st_ = (k == 0)
                        sp = (k == NG - 1) and not has_pjb
                        for n in range(2):
                            sl = slice(n * 512, (n + 1) * 512)
                            nc.tensor.matmul(yp[n][:], lhs, pjw[k][:, sl],
                                             start=st_, stop=sp)
                    if has_pjb:
                        for n in range(2):
                            sl = slice(n * 512, (n + 1) * 512)
                            nc.tensor.matmul(yp[n][:], ones1r[:],
                                             pjb[:, sl],
                                             start=False, stop=True)
                    yraw, s1h, s2h = [], [], []
                    for n in range(2):
                        yr = pe_p.tile([P, 512], F32, tag=f"yraw{n}",
                                       name=f"yraw{t_i}_{n}")
                        s1n = pe_p.tile([P, 1], F32, tag=f"s1{n}",
                                        name=f"s1_{t_i}_{n}")
                        nc.scalar.activation(yr[:], yp[n][:], ACTF.Copy,
                                             accum_out=s1n[:])
                        yraw.append(yr)
                        s1h.append(s1n)
                    for n in range(2):
                        sc4 = pe_p.tile([P, 512], F32, tag="sc4",
                                        name=f"sc4_{t_i}_{n}", bufs=1)
                        s2n = pe_p.tile([P, 1], F32, tag=f"s2{n}",
                                        name=f"s2_{t_i}_{n}")
                        nc.scalar.activation(sc4[:], yraw[n][:], ACTF.Square,
                                             scale=1.0 / 32.0,
                                             accum_out=s2n[:])
                        s2h.append(s2n)
                    s1t = pe_p.tile([P, 1], F32, tag="s1t", name=f"s1t{t_i}")
                    nc.vector.tensor_tensor(s1t[:], s1h[0][:], s1h[1][:],
                                            op=ALU.add)
                    m1 = pe_p.tile([P, 1], F32, tag="m1", name=f"m1_{t_i}")
                    nc.vector.tensor_scalar(m1[:], s1t[:], 1.0 / C, None,
                                            op0=ALU.mult)
                    s2t = pe_p.tile([P, 1], F32, tag="s2t", name=f"s2t{t_i}")
                    nc.vector.tensor_tensor(s2t[:], s2h[0][:], s2h[1][:],
                                            op=ALU.add)
                    msq = pe_p.tile([P, 1], F32, tag="msq", name=f"msq{t_i}")
                    nc.vector.tensor_tensor(msq[:], m1[:], m1[:],
                                            op=ALU.mult)
                    var4 = pe_p.tile([P, 1], F32, tag="var4",
                                     name=f"var4_{t_i}")
                    nc.vector.tensor_tensor(var4[:], s2t[:], msq[:],
                                            op=ALU.subtract)
                    sd4 = pe_p.tile([P, 1], F32, tag="sd4",
                                    name=f"sd4_{t_i}")
                    nc.scalar.activation(sd4[:], var4[:], ACTF.Sqrt,
                                         bias=eps128[:])
                    rstd4 = pe_p.tile([P, 1], F32, tag="rstd4",
                                      name=f"rstd4_{t_i}")
                    nc.vector.reciprocal(rstd4[:], sd4[:])
                    yout = pe_p.tile([P, C], F32, tag="yout",
                                     name=f"yout{t_i}")
                    for n in range(2):
                        sl = slice(n * 512, (n + 1) * 512)
                        tn = pe_p.tile([P, 512], F32, tag=f"tn{n}",
                                       name=f"tn{t_i}_{n}", bufs=1)
                        nc.vector.tensor_scalar(tn[:], yraw[n][:], m1[:],
                                                rstd4[:], op0=ALU.subtract,
                                                op1=ALU.mult)
                        if has_ln_g:
                            nc.vector.tensor_tensor(tn[:], tn[:], lgr[:, sl],
                                                    op=ALU.mult)
                        if has_ln_b:
                            nc.vector.tensor_tensor(tn[:], tn[:], lbr[:, sl],
                                                    op=ALU.add)
                        if pg == NPG - 1:
                            # drain tail: vector is idle, gpsimd is slow
                            nc.vector.tensor_tensor(yout[:, sl], tn[:],
                                                    xa[:, sl], op=ALU.add)
                        else:
                            nc.gpsimd.tensor_tensor(yout[:, sl], tn[:],
                                                    xa[:, sl], op=ALU.add)
                    nc.sync.dma_start(y_d.ap()[t_i * P:(t_i + 1) * P, :],
                                      yout[:])

            # software pipeline, distance 2: C(pg) | D(pg-1) | E(pg-2).
            # Static interleave: E's dense matmul bursts fill the PE queue
            # while D's activation chain and C's LN chain are in flight.
            cts, ots = {}, {}
            for pg in range(NPG + 2):
                if pg < NPG:
                    cts[pg] = make_ct(pg)
                if 1 <= pg <= NPG:
                    ots[pg - 1] = make_ot(pg - 1)

                def d_h1(hi):
                    if 1 <= pg <= NPG:
                        return D_h1(pg - 1, hi, ct_accessor(cts[pg - 1]))
                    return None

                def d_o2(hi, st_):
                    if st_ is not None:
                        D_o2(pg - 1, hi, st_[0], st_[1], ots[pg - 1])

                def c_unit(i):
                    if pg < NPG:
                        C_unit(pg, i, cts[pg])

                def e_tt(i):
                    if pg >= 2:
                        E_tt(pg - 2, i, ots[pg - 2])

                stA = d_h1(0)
                e_tt(0)
                c_unit(0)
                d_o2(0, stA)
                e_tt(1)
                c_unit(1)
                stB = d_h1(1)
                e_tt(2)
                c_unit(2)
                d_o2(1, stB)
                e_tt(3)
                c_unit(3)
                if pg >= 2:
                    del cts[pg - 2], ots[pg - 2]

    nc.compile()
    return nc


_CACHE = {}


def _get_program(alpha, flags):
    key = (alpha, flags)
    if key not in _CACHE:
        _CACHE[key] = _build(NCORES, alpha, *flags)
    return _CACHE[key]


def _bf16(a):
    return np.ascontiguousarray(a.astype(ml_dtypes.bfloat16))


def prepare(inputs):
    """Compute flags + the per-core input maps (host-side prep)."""
    x = np.ascontiguousarray(np.asarray(inputs["x"], np.float32))
    mark_W = np.asarray(inputs["mark_W"], np.float32)
    mark_b = np.asarray(inputs["mark_b"], np.float32)
    gate_W = np.asarray(inputs["gate_W"], np.float32)
    gate_b = np.asarray(inputs["gate_b"], np.float32)
    carry_g = np.asarray(inputs["carry_g"], np.float32)
    carry_b = np.asarray(inputs["carry_b"], np.float32)
    card_g = np.asarray(inputs["card_g"], np.float32)
    card_b = np.asarray(inputs["card_b"], np.float32)
    W1 = np.asarray(inputs["W1"], np.float32)
    b1 = np.asarray(inputs["b1"], np.float32)
    alpha = float(np.asarray(inputs["alpha"]))
    W2 = np.asarray(inputs["W2"], np.float32)
    b2 = np.asarray(inputs["b2"], np.float32)
    proj_W = np.asarray(inputs["proj_W"], np.float32)
    proj_b = np.asarray(inputs["proj_b"], np.float32)
    ln_g = np.asarray(inputs["ln_g"], np.float32)
    ln_b = np.asarray(inputs["ln_b"], np.float32)

    W1x = np.concatenate([W1[:D, :], W1[:D, :]], 0)
    W1c0 = card_g[:, None] * W1[D:, :]
    W1c = np.concatenate([W1c0, W1c0], 0)
    b1f = (b1 + card_b @ W1[D:, :]).astype(np.float32)
    # fold b2 into the proj bias: y = ho @ proj_W + proj_b, ho = .. + b2
    row = np.tile(b2, H).astype(np.float32)
    pjb_eff = (proj_b + row @ proj_W).astype(np.float32)

    stm = np.zeros((P, P), np.float32)
    for i in range(1, P):
        stm[i - 1, i] = 1.0

    has_carry_gb = bool(np.any(carry_g != 1.0) or np.any(carry_b != 0.0))
    flags = (bool(np.any(mark_b)), bool(np.any(gate_b)),
             bool(np.any(pjb_eff)), has_carry_gb,
             bool(np.any(ln_g != 1.0)), bool(np.any(ln_b)))

    common = {
        "mkw": _bf16(mark_W), "gtw": _bf16(gate_W), "pjw": _bf16(proj_W),
        "mkb": _bf16(mark_b[None, :]), "gtb": _bf16(gate_b[None, :]),
        "pjb": _bf16(pjb_eff[None, :]),
        "w1x": _bf16(W1x), "w1c": _bf16(W1c), "b1c": b1f[:, None],
        "w2": _bf16(W2), "w2a": _bf16(alpha * W2),
        "ut": _bf16(np.triu(np.ones((P, P), np.float32))),
        "st": np.ascontiguousarray(stm.astype(ml_dtypes.bfloat16)),
        "l0": _bf16(np.triu(np.ones((NCH, NCH), np.float32), k=1)),
        "onesr": _bf16(np.ones((1, P), np.float32)),
        "selb": _bf16(np.concatenate(
            [np.eye(NCH, dtype=np.float32)[:, j:j + 1] * np.ones((1, P))
             for j in range(NCH)], axis=1)),
        "cgr": np.tile(carry_g[None, :], (NCH, 1)).astype(np.float32),
        "cbr": np.tile(carry_b[None, :], (NCH, 1)).astype(np.float32),
        "lgr": np.tile(ln_g[None, :], (P, 1)).astype(np.float32),
        "lbr": np.tile(ln_b[None, :], (P, 1)).astype(np.float32),
    }
    in_maps = []
    for c in range(NCORES):
        b, half = c // 2, c % 2
        xs = x[b, half * R:(half + 1) * R, :]
        m = dict(common)
        m["xn"] = np.ascontiguousarray(xs)
        m["xt"] = _bf16(xs.T)
        m["segm"] = np.array([[1.0 - half]], np.float32)
        m["usem"] = np.array([[float(half)]], np.float32)
        in_maps.append(m)
    return alpha, flags, in_maps


def kernel(**inputs):
    alpha, flags, in_maps = prepare(inputs)
    nc = _get_program(alpha, flags)
    res = run_bass_kernel_spmd(nc, in_maps, list(range(NCORES)))
    out = np.empty((B, T, C), np.float32)
    for c in range(NCORES):
        b, half = c // 2, c % 2
        out[b, half * R:(half + 1) * R, :] = res.results[c]["y"]
    return out
